# revision 2
# baseline (speedup 1.0000x reference)
"""Trainium2 Bass kernel for multi-head attention (B=4, N=2048, DIM=1024, H=16, DH=64).

Sharding: 8 cores = 4 batches x 2 query-halves. Each core computes, for its
batch b and query rows i in [half*1024, half*1024+1024):
  qkv projection (q only for its i-half, k/v for all 2048 rows),
  scores^T = k_h^T(row-tiled pairs) @ q_h, softmax via exp + ones-column
  denominator folded into the AV matmul, out = attn@v (transposed form),
  and the full output projection for its rows. Outputs are disjoint.
"""

import numpy as np
import ml_dtypes

import concourse.bass as bass
import concourse.tile as tile
from concourse import bacc, mybir
from concourse import bass_utils

B, N, DIM = 4, 2048, 1024
HEADS, DH = 16, 64
INNER = HEADS * DH
SCALE = DH ** -0.5
NCORES = 8
IH = N // 2          # query rows per core (i-half)
BF16 = mybir.dt.bfloat16
F32 = mybir.dt.float32

_CACHE = {}


def _build_program():
    nc = bacc.Bacc("TRN2", target_bir_lowering=False, debug=False)

    xT_d = nc.dram_tensor("xT", [DIM, N], BF16, kind="ExternalInput")
    xTq_d = nc.dram_tensor("xTq", [DIM, IH], BF16, kind="ExternalInput")
    wqkv_d = nc.dram_tensor("w_qkv", [DIM, 3 * INNER], BF16, kind="ExternalInput")
    wout_d = nc.dram_tensor("w_out_perm", [DH, HEADS, DIM], BF16, kind="ExternalInput")
    bout_d = nc.dram_tensor("b_out", [DIM], F32, kind="ExternalInput")
    out_d = nc.dram_tensor("out", [IH, DIM], F32, kind="ExternalOutput")

    KT = DIM // 128          # 8 contraction tiles for projections
    NT = N // 128            # 16 j tiles
    ES = INNER // 128        # 8 e-slices for q or k
    with tile.TileContext(nc) as tc:
        _emit(tc, nc, xT_d, xTq_d, wqkv_d, wout_d, bout_d, out_d, KT, NT, ES)
    nc.compile()
    return nc


def _emit(tc, nc, xT_d, xTq_d, wqkv_d, wout_d, bout_d, out_d, KT, NT, ES):
    from contextlib import ExitStack

    xT_r = xT_d.ap().rearrange("(t p) n -> p t n", p=128)       # [128, 8, 2048]
    xTq_r = xTq_d.ap().rearrange("(t p) n -> p t n", p=128)     # [128, 8, 1024]
    w_r = wqkv_d.ap().rearrange("(t p) e -> p t e", p=128)      # [128, 8, 3072]

    bap = bout_d.ap()
    bias_bcast = bass.AP(tensor=bap.tensor, offset=bap.offset,
                         ap=[[0, 128]] + [list(d) for d in bap.ap])

    with ExitStack() as ctx:
        consts = ctx.enter_context(tc.tile_pool(name="consts", bufs=1))
        qkv_out = ctx.enter_context(tc.tile_pool(name="qkv_out", bufs=1))

        bias_sb = consts.tile([128, DIM], F32)
        nc.sync.dma_start(out=bias_sb, in_=bias_bcast)
        ones_sb = consts.tile([128, DH], F32)
        nc.vector.memset(ones_sb, 1.0)

        qT = qkv_out.tile([128, ES, IH], BF16)          # [e-part, e-tile, i]
        kT = qkv_out.tile([128, ES, N], BF16)           # [e-part, e-tile, j]
        v_sb = qkv_out.tile([128, NT, HEADS, DH + 1], BF16)  # [j-part, j-tile, h, d+1]
        nc.vector.memset(v_sb[:, :, :, DH], 1.0)

        # ---------------- phase 1: projections ----------------
        with tc.tile_pool(name="p1_x", bufs=1) as p1x, \
             tc.tile_pool(name="p1_w", bufs=2) as p1w, \
             tc.tile_pool(name="p1_ps", bufs=3, space="PSUM") as p1ps:
            xT_sb = p1x.tile([128, KT, N], BF16)
            nc.sync.dma_start(out=xT_sb, in_=xT_r)
            xTq_sb = p1x.tile([128, KT, IH], BF16)
            nc.sync.dma_start(out=xTq_sb, in_=xTq_r)

            for g in range(6):                      # e-groups of 512 cols
                wg = p1w.tile([128, KT, 512], BF16)
                nc.sync.dma_start(out=wg, in_=w_r[:, :, 512 * g:512 * (g + 1)])
                if g < 2:
                    # q columns: out qT[e, i] for e-slices 4g..4g+3
                    for s4 in range(4):
                        s = 4 * g + s4
                        ps = p1ps.tile([128, IH], F32)
                        for c in range(IH // 512):
                            for k in range(KT):
                                nc.tensor.matmul(
                                    ps[:, 512 * c:512 * (c + 1)],
                                    wg[:, k, 128 * s4:128 * (s4 + 1)],
                                    xTq_sb[:, k, 512 * c:512 * (c + 1)],
                                    start=(k == 0), stop=(k == KT - 1))
                        nc.vector.tensor_copy(out=qT[:, s, :], in_=ps)
                elif g < 4:
                    # k columns: out kT[e, j] for e-slices 4(g-2)..+3
                    for s4 in range(4):
                        s = 4 * (g - 2) + s4
                        for half in range(2):
                            ps = p1ps.tile([128, IH], F32)
                            for c in range(IH // 512):
                                for k in range(KT):
                                    nc.tensor.matmul(
                                        ps[:, 512 * c:512 * (c + 1)],
                                        wg[:, k, 128 * s4:128 * (s4 + 1)],
                                        xT_sb[:, k, IH * half + 512 * c:IH * half + 512 * (c + 1)],
                                        start=(k == 0), stop=(k == KT - 1))
                            nc.vector.tensor_copy(
                                out=kT[:, s, IH * half:IH * (half + 1)], in_=ps)
                else:
                    # v columns: heads 8*(g-4) .. +8 ; out v[n, e]
                    ec = g - 4
                    for t in range(NT):
                        ps = p1ps.tile([128, 512], F32)
                        for k in range(KT):
                            nc.tensor.matmul(
                                ps, xT_sb[:, k, 128 * t:128 * (t + 1)],
                                wg[:, k, :],
                                start=(k == 0), stop=(k == KT - 1))
                        nc.vector.tensor_copy(
                            out=v_sb[:, t, 8 * ec:8 * (ec + 1), 0:DH],
                            in_=ps.rearrange("p (h d) -> p h d", h=8))

        # ---------------- phase 2: attention ----------------
        attn_out = ctx.enter_context(tc.tile_pool(name="attn_out", bufs=1))
        aoT = attn_out.tile([DH, HEADS, IH], BF16)      # [d, h, i] all heads at parts 0-63

        with tc.tile_pool(name="attnT", bufs=2) as atp, \
             tc.tile_pool(name="rcp", bufs=2) as rcp, \
             tc.tile_pool(name="bcs", bufs=2) as bcsp, \
             tc.tile_pool(name="ps_sc", bufs=2, space="PSUM") as ps_sc, \
             tc.tile_pool(name="ps_av", bufs=2, space="PSUM") as ps_av, \
             tc.tile_pool(name="ps_bc", bufs=2, space="PSUM") as ps_bc:
            for s in range(ES):                 # head pair (2s, 2s+1)
                at0 = atp.tile([128, NT, IH], BF16, tag="at")
                at1 = atp.tile([128, NT, IH], BF16, tag="at")
                ats = [at0, at1]
                for t in range(NT):
                    for p in range(2):          # head half within pair
                        h = 2 * s + p
                        pb = 64 * p
                        sc = ps_sc.tile([128, IH], F32, tag="sc")
                        for c in range(IH // 512):
                            nc.tensor.matmul(
                                sc[:, 512 * c:512 * (c + 1)],
                                kT[pb:pb + 64, s, 128 * t:128 * (t + 1)],
                                qT[pb:pb + 64, s, 512 * c:512 * (c + 1)],
                                start=True, stop=True,
                                tile_position=(pb, 0))
                        nc.scalar.activation(
                            out=ats[p][:, t, :], in_=sc,
                            func=mybir.ActivationFunctionType.Exp, scale=SCALE)
                for p in range(2):
                    h = 2 * s + p
                    for c in range(IH // 512):
                        av = ps_av.tile([DH + 1, 512], F32, tag="av")
                        for t in range(NT):
                            nc.tensor.matmul(
                                av, v_sb[:, t, h, :],
                                ats[p][:, t, 512 * c:512 * (c + 1)],
                                start=(t == 0), stop=(t == NT - 1))
                        rc = rcp.tile([128, 512], F32, tag="rc")
                        nc.vector.reciprocal(out=rc[DH:DH + 1, :], in_=av[DH:DH + 1, :])
                        bc = ps_bc.tile([DH, 512], F32, tag="bc")
                        nc.tensor.matmul(
                            bc, ones_sb[DH:DH + 1, :], rc[DH:DH + 1, :],
                            start=True, stop=True, tile_position=(64, 0))
                        bcs = bcsp.tile([DH, 512], F32, tag="bcs")
                        nc.vector.tensor_copy(out=bcs, in_=bc)
                        nc.vector.tensor_mul(
                            out=aoT[:, h, 512 * c:512 * (c + 1)],
                            in0=av[0:DH, :], in1=bcs)

        # ---------------- phase 3: output projection ----------------
        with tc.tile_pool(name="p3_w", bufs=1) as p3w, \
             tc.tile_pool(name="p3_st", bufs=2) as p3st, \
             tc.tile_pool(name="ps_out", bufs=2, space="PSUM") as ps_out:
            wo = p3w.tile([DH, HEADS, DIM], BF16)
            nc.sync.dma_start(out=wo, in_=wout_d.ap())
            for ns in range(IH // 128):
                po = ps_out.tile([128, DIM], F32)
                for c in range(DIM // 512):
                    for h in range(HEADS):
                        nc.tensor.matmul(
                            po[:, 512 * c:512 * (c + 1)],
                            aoT[:, h, 128 * ns:128 * (ns + 1)],
                            wo[:, h, 512 * c:512 * (c + 1)],
                            start=(h == 0), stop=(h == HEADS - 1))
                st = p3st.tile([128, DIM], F32)
                nc.vector.tensor_add(out=st, in0=po, in1=bias_sb)
                nc.sync.dma_start(out=out_d.ap()[128 * ns:128 * (ns + 1), :], in_=st)


def get_program():
    if "nc" not in _CACHE:
        _CACHE["nc"] = _build_program()
    return _CACHE["nc"]


def make_in_maps(x, w_qkv, w_out, b_out):
    bf = ml_dtypes.bfloat16
    w_qkv_b = np.ascontiguousarray(w_qkv, np.float32).astype(bf)
    w_out_p = np.ascontiguousarray(
        np.asarray(w_out, np.float32).reshape(HEADS, DH, DIM).transpose(1, 0, 2)
    ).astype(bf)
    b_out_f = np.ascontiguousarray(b_out, np.float32)
    in_maps = []
    for core in range(NCORES):
        b, half = core // 2, core % 2
        xT = np.ascontiguousarray(np.asarray(x[b], np.float32).T).astype(bf)
        in_maps.append({
            "xT": xT,
            "xTq": np.ascontiguousarray(xT[:, IH * half:IH * (half + 1)]),
            "w_qkv": w_qkv_b,
            "w_out_perm": w_out_p,
            "b_out": b_out_f,
        })
    return in_maps


def kernel(x, w_qkv, w_out, b_out):
    nc = get_program()
    in_maps = make_in_maps(x, w_qkv, w_out, b_out)
    res = bass_utils.run_bass_kernel_spmd(nc, in_maps, core_ids=list(range(NCORES)))
    out = np.empty((B, N, DIM), np.float32)
    for core in range(NCORES):
        b, half = core // 2, core % 2
        out[b, IH * half:IH * (half + 1), :] = res.results[core]["out"]
    return out
